# revision 35
# baseline (speedup 1.0000x reference)
"""Two-layer GAT (8-head + 1-head) Trainium2 Bass kernel, 8-way node-sharded.

Strategy (per core c, owning row block I_c of R = N/8 nodes), layer 1:
  * Softmax over neighbors j is invariant to per-row (per-i) scaling, so
    P[j, i] ~ adjT[j, i] * max(exp((1-a)fs_i + fd_j), exp(a * fd_j))
    using exp(leakyrelu(z)) = max(exp(z), exp(a z)) and dropping the
    exp(a fs_i) row factor.  exp is evaluated by ScalarE directly as
    t1 = Exp(0.8 * fsb + fd_j) (fsb = broadcast f_src rows, fd as the
    per-partition bias); the N x R attention field then needs a SINGLE
    fused DVE op per tile: p = (t1 max vd_j) * adjT.
  * Scores live in TRANSPOSED layout [j (partition), i (free)] so both
    the aggregation out^T[o, i] = sum_j h[j, o] P[j, i] and the softmax
    denominator Z contract over the partition dim.  Z rides an all-ones
    stationary matmul, landing broadcast across all 128 partitions so
    normalization needs no further broadcast.
  * h = x @ W1 for all nodes is computed locally (replicated), with the
    fd columns falling out of the same pass via a folded [512, 8]
    matmul.
  * Layer-2 inputs h2 = h1 @ W2 (+ f_src2/f_dst2 via folded W2 columns)
    are tiny ([N, 18]); each core computes its own block and an
    AllGather distributes them; layer 2 repeats the same scheme with a
    single head.
"""

import sys

sys.path.insert(0, "/opt/trn_rl_repo")

import numpy as np
import ml_dtypes

N = 4096
F_IN = 512
H1 = 8
D1 = 128
F1 = 1024          # H1 * D1
D2 = 16
NCORES = 8
R = N // NCORES    # rows (nodes) per core
NCH = N // 128     # j-chunks of 128
NFC = F_IN // 128  # f chunks
ALPHA = 0.2
BIG = 1e38         # mask scale: adjT ships as {0, BIG}; mask = min(u, adjT)
WIDE_MM = False    # single matmul spanning 2 PSUM banks fails neuronx codegen

_BUILD_CACHE = {}


def _build_nc():
    import concourse.bacc as bacc
    import concourse.tile as tile
    import concourse.mybir as mybir

    FP32 = mybir.dt.float32
    BF16 = mybir.dt.bfloat16
    AF = mybir.ActivationFunctionType
    OP = mybir.AluOpType
    AX = mybir.AxisListType

    nc = bacc.Bacc(num_devices=NCORES)

    # ---- I/O -------------------------------------------------------------
    xT_d = nc.dram_tensor("xT", [F_IN, N], BF16, kind="ExternalInput")
    xTo_d = nc.dram_tensor("xTo", [F_IN, R], BF16, kind="ExternalInput")
    W1_d = nc.dram_tensor("W1f", [F_IN, F1], BF16, kind="ExternalInput")
    adr_d = nc.dram_tensor("adstrow", [1, F1], BF16, kind="ExternalInput")
    wsn_d = nc.dram_tensor("wsn", [F_IN, 8], BF16, kind="ExternalInput")
    adjT_d = nc.dram_tensor("adjT", [N, R], BF16, kind="ExternalInput")
    onesb_d = nc.dram_tensor("onesb", [128, 128], BF16, kind="ExternalInput")
    idb_d = nc.dram_tensor("idb", [128, 128], BF16, kind="ExternalInput")
    idf_d = nc.dram_tensor("idf16", [16, 16], FP32, kind="ExternalInput")
    W2a_d = nc.dram_tensor("W2a", [F1, 18], BF16, kind="ExternalInput")
    out_d = nc.dram_tensor("out", [R, D2], FP32, kind="ExternalOutput")

    with tile.TileContext(nc) as tc:
        with (
            tc.tile_pool(name="const", bufs=1) as cpool,
            tc.tile_pool(name="dram", bufs=1, space="DRAM") as dpool,
        ):
            # ---- resident SBUF tensors ----------------------------------
            W1_sb = cpool.tile([128, NFC * F1], BF16, tag="W1")
            adr_sb = cpool.tile([1, F1], BF16, tag="adr")
            adb_sb = cpool.tile([128, F1], BF16, tag="adb")
            wsn_sb = cpool.tile([128, NFC * 8], BF16, tag="wsn")
            xo_sb = cpool.tile([128, NFC * R], BF16, tag="xo")
            onesb_sb = cpool.tile([128, 128], BF16, tag="onesb")
            idb_sb = cpool.tile([128, 128], BF16, tag="idb")
            idf_sb = cpool.tile([16, 16], FP32, tag="idf")
            W2a_sb = cpool.tile([128, (F1 // 128) * 18], BF16, tag="W2a")
            adjT_sb = cpool.tile([128, NCH * R], BF16, tag="adjT")

            h_sb = cpool.tile([128, NCH * F1], BF16, tag="h")
            fsb = cpool.tile([128, H1 * R], BF16, tag="fsb")
            abc = cpool.tile([128, H1 * R], BF16, tag="abc")
            fsx = cpool.tile([128, 2 * R], BF16, tag="fsx")
            fd_sb = cpool.tile([128, NCH * 8], BF16, tag="fd")
            efd_sb = cpool.tile([128, NCH * 8], FP32, tag="efd")
            vd_sb = cpool.tile([128, NCH * 8], FP32, tag="vd")
            h1T = cpool.tile([128, H1 * R], BF16, tag="h1T")
            h2all_sb = cpool.tile([128, NCH * 18], BF16, tag="h2all")
            fs2row = cpool.tile([1, R], FP32, tag="fs2row")
            fsb2 = cpool.tile([128, R], BF16, tag="fsb2")
            fd2_sb = cpool.tile([128, NCH], FP32, tag="fd2")
            vd2_sb = cpool.tile([128, NCH], FP32, tag="vd2")

            h2loc = dpool.tile([R, 18], BF16, tag="h2loc")
            h2all_d = dpool.tile([N, 18], BF16, tag="h2all", addr_space="Shared")

            # ---- input DMAs, ordered so phase B can start ASAP ----------
            for fc in range(NFC):
                nc.sync.dma_start(
                    W1_sb[:, fc * F1:(fc + 1) * F1],
                    W1_d[fc * 128:(fc + 1) * 128, :],
                )
            nc.sync.dma_start(adr_sb[:], adr_d[:])
            nc.sync.dma_start(onesb_sb[:], onesb_d[:])

            # =============================================================
            # Phase B: h = x @ W1 (all nodes) + fd columns; streams xT in
            # =============================================================
            with tc.tile_pool(name="xres", bufs=1) as xrp:
                x_sb = xrp.tile([128, NFC * N], BF16, tag="x")
                # quarter-column DMAs, q-major so early jt chunks land first
                for q in range(4):
                    for fc in range(NFC):
                        nc.sync.dma_start(
                            x_sb[:, fc * N + q * 1024:fc * N + (q + 1) * 1024],
                            xT_d[fc * 128:(fc + 1) * 128,
                                 q * 1024:(q + 1) * 1024],
                        )
                # the rest of the inputs, roughly in order of first use
                for fc in range(NFC):
                    nc.sync.dma_start(
                        xo_sb[:, fc * R:(fc + 1) * R],
                        xTo_d[fc * 128:(fc + 1) * 128, :],
                    )
                    nc.sync.dma_start(
                        wsn_sb[:, fc * 8:(fc + 1) * 8],
                        wsn_d[fc * 128:(fc + 1) * 128, :],
                    )
                for c in range(NCH):
                    nc.sync.dma_start(
                        adjT_sb[:, c * R:(c + 1) * R],
                        adjT_d[c * 128:(c + 1) * 128, :],
                    )
                nc.sync.dma_start(idb_sb[:], idb_d[:])
                nc.sync.dma_start(idf_sb[:], idf_d[:])
                for g in range(F1 // 128):
                    nc.sync.dma_start(
                        W2a_sb[:, g * 18:(g + 1) * 18],
                        W2a_d[g * 128:(g + 1) * 128, :],
                    )

                # broadcast a_dst row across partitions (for fd-from-h)
                with tc.tile_pool(name="padb", bufs=1, space="PSUM") as padp:
                    pad = padp.tile([128, F1], FP32, tag="pad")
                    for half in range(2):
                        nc.tensor.matmul(
                            pad[:, half * 512:(half + 1) * 512],
                            onesb_sb[0:1, :],
                            adr_sb[0:1, half * 512:(half + 1) * 512],
                            start=True, stop=True,
                        )
                    nc.scalar.activation(adb_sb[:], pad[:], AF.Copy)

                with (
                    tc.tile_pool(name="ph", bufs=4, space="PSUM") as php,
                    tc.tile_pool(name="hp", bufs=3) as hpp,
                ):
                    for jt in range(NCH):
                        ph = php.tile([128, F1], FP32, tag="ph")
                        for fc in range(NFC):
                            lhs = x_sb[:, fc * N + jt * 128:fc * N + (jt + 1) * 128]
                            st, sp = fc == 0, fc == NFC - 1
                            if WIDE_MM:
                                nc.tensor.matmul(
                                    ph[:], lhs,
                                    W1_sb[:, fc * F1:(fc + 1) * F1],
                                    start=st, stop=sp,
                                )
                            else:
                                nc.tensor.matmul(
                                    ph[:, 0:512], lhs,
                                    W1_sb[:, fc * F1:fc * F1 + 512],
                                    start=st, stop=sp,
                                )
                                nc.tensor.matmul(
                                    ph[:, 512:F1], lhs,
                                    W1_sb[:, fc * F1 + 512:(fc + 1) * F1],
                                    start=st, stop=sp,
                                )
                        nc.scalar.activation(
                            h_sb[:, jt * F1:jt * F1 + 512], ph[:, 0:512], AF.Copy
                        )
                        nc.scalar.activation(
                            h_sb[:, jt * F1 + 512:(jt + 1) * F1], ph[:, 512:F1],
                            AF.Copy,
                        )
                        # fd[j, g] = sum_o h[j, g, o] * a_dst[g, o]
                        prod = hpp.tile([128, F1], BF16, tag="prod")
                        nc.vector.tensor_mul(
                            prod[:], h_sb[:, jt * F1:(jt + 1) * F1], adb_sb[:]
                        )
                        with nc.allow_low_precision("fd accumulated in bf16"):
                            nc.vector.tensor_reduce(
                                fd_sb[:, jt * 8:(jt + 1) * 8],
                                prod[:].rearrange("p (g o) -> p g o", o=D1),
                                AX.X, OP.add,
                            )
                        nc.scalar.activation(
                            vd_sb[:, jt * 8:(jt + 1) * 8],
                            fd_sb[:, jt * 8:(jt + 1) * 8], AF.Exp,
                            scale=ALPHA,
                        )
                        nc.scalar.activation(
                            efd_sb[:, jt * 8:(jt + 1) * 8],
                            fd_sb[:, jt * 8:(jt + 1) * 8], AF.Exp,
                        )

            # =============================================================
            # Phase A: own-block f_src -> broadcast rows fsb
            # =============================================================
            with (
                tc.tile_pool(name="pfs", bufs=1, space="PSUM") as pfsp,
                tc.tile_pool(name="pab", bufs=2, space="PSUM") as pabp,
                tc.tile_pool(name="a8p", bufs=1) as a8p,
            ):
                fsT8 = pfsp.tile([8, R], FP32, tag="fs8")
                for fc in range(NFC):
                    nc.tensor.matmul(
                        fsT8[:],
                        wsn_sb[:, fc * 8:(fc + 1) * 8],
                        xo_sb[:, fc * R:(fc + 1) * R],
                        start=fc == 0, stop=fc == NFC - 1,
                    )
                fs8 = a8p.tile([8, R], BF16, tag="fs8s")
                nc.scalar.activation(fs8[:], fsT8[:], AF.Copy)
                for g in range(H1):
                    q, hf = g % 4, g // 4
                    nc.sync.dma_start(
                        fsx[32 * q:32 * q + 1, hf * R:(hf + 1) * R],
                        fs8[g:g + 1, :],
                    )
                for g in range(H1):
                    q, hf = g % 4, g // 4
                    pb = pabp.tile([128, R], FP32, tag="pab")
                    nc.tensor.matmul(
                        pb[:],
                        onesb_sb[32 * q:32 * q + 1, :],
                        fsx[32 * q:32 * q + 1, hf * R:(hf + 1) * R],
                        start=True, stop=True, tile_position=(32 * q, 0),
                    )
                    nc.scalar.activation(
                        fsb[:, g * R:(g + 1) * R], pb[:], AF.Copy
                    )
                    nc.scalar.activation(
                        abc[:, g * R:(g + 1) * R], pb[:], AF.Exp,
                        scale=1.0 - ALPHA,
                    )

            # =============================================================
            # Phase C: layer-1 attention, 2 heads (one pair) per pass
            # =============================================================
            with (
                tc.tile_pool(name="acc", bufs=2, space="PSUM") as acc,
                tc.tile_pool(name="sc", bufs=4) as spool,
                tc.tile_pool(name="nrm", bufs=2) as npool,
            ):
                for p in range(4):
                    g0, g1 = 2 * p, 2 * p + 1
                    po0 = acc.tile([128, R], FP32, tag="po0", name="po0")
                    po1 = acc.tile([128, R], FP32, tag="po1", name="po1")
                    pz = acc.tile([128, 2 * R], FP32, tag="pz", name="pz")
                    for c in range(NCH):
                        t1 = spool.tile([128, 2 * R], BF16, tag="t1")
                        up = spool.tile([128, 2 * R], BF16, tag="up")
                        pp = spool.tile([128, 2 * R], BF16, tag="pp")
                        for k, g in ((0, g0), (1, g1)):
                            if (2 * c + k) % 3 == 0:
                                # ACT path: exp(0.8 fs + fd) then max on DVE
                                nc.scalar.activation(
                                    t1[:, k * R:(k + 1) * R],
                                    fsb[:, g * R:(g + 1) * R],
                                    AF.Exp,
                                    bias=fd_sb[:, c * 8 + g:c * 8 + g + 1],
                                    scale=1.0 - ALPHA,
                                )
                                nc.vector.tensor_scalar_max(
                                    up[:, k * R:(k + 1) * R],
                                    t1[:, k * R:(k + 1) * R],
                                    vd_sb[:, c * 8 + g:c * 8 + g + 1],
                                )
                            else:
                                # DVE path: max(exp(0.8 fs) * exp(fd), vd)
                                nc.vector.tensor_scalar(
                                    up[:, k * R:(k + 1) * R],
                                    abc[:, g * R:(g + 1) * R],
                                    efd_sb[:, c * 8 + g:c * 8 + g + 1],
                                    vd_sb[:, c * 8 + g:c * 8 + g + 1],
                                    op0=OP.mult, op1=OP.max,
                                )
                        nc.vector.tensor_mul(
                            pp[:].rearrange("p (k f) -> p k f", f=R),
                            up[:].rearrange("p (k f) -> p k f", f=R),
                            adjT_sb[:, c * R:(c + 1) * R]
                            .unsqueeze(1)
                            .broadcast_to([128, 2, R]),
                        )
                        nc.tensor.matmul(
                            po0[:],
                            h_sb[:, c * F1 + g0 * D1:c * F1 + (g0 + 1) * D1],
                            pp[:, 0:R],
                            start=c == 0, stop=c == NCH - 1,
                        )
                        nc.tensor.matmul(
                            po1[:],
                            h_sb[:, c * F1 + g1 * D1:c * F1 + (g1 + 1) * D1],
                            pp[:, R:2 * R],
                            start=c == 0, stop=c == NCH - 1,
                        )
                        if WIDE_MM:
                            nc.tensor.matmul(
                                pz[:], onesb_sb[:], pp[:],
                                start=c == 0, stop=c == NCH - 1,
                            )
                        else:
                            nc.tensor.matmul(
                                pz[:, 0:R], onesb_sb[:], pp[:, 0:R],
                                start=c == 0, stop=c == NCH - 1,
                            )
                            nc.tensor.matmul(
                                pz[:, R:2 * R], onesb_sb[:], pp[:, R:2 * R],
                                start=c == 0, stop=c == NCH - 1,
                            )
                    # normalize + ELU -> h1^T (bf16)
                    zr = npool.tile([128, 2 * R], FP32, tag="zr")
                    nc.vector.reciprocal_approx_fast(zr[:], pz[:])
                    for k, g, po in ((0, g0, po0), (1, g1, po1)):
                        pre = npool.tile([128, R], FP32, tag="pre")
                        nc.vector.tensor_mul(
                            pre[:], po[:], zr[:, k * R:(k + 1) * R]
                        )
                        r = npool.tile([128, R], FP32, tag="r")
                        nc.scalar.activation(r[:], pre[:], AF.Relu, scale=-1.0)
                        t = npool.tile([128, R], FP32, tag="t")
                        nc.scalar.activation(t[:], r[:], AF.Exp, scale=-1.0)
                        nc.vector.scalar_tensor_tensor(
                            h1T[:, g * R:(g + 1) * R], t[:], -1.0, pre[:],
                            op0=OP.add, op1=OP.max,
                        )

            # =============================================================
            # Phase D: layer-2 transform + AllGather of [h2 | fs2 | fd2]
            # =============================================================
            with (
                tc.tile_pool(name="p2", bufs=2, space="PSUM") as p2p,
                tc.tile_pool(name="p2t", bufs=2, space="PSUM") as p2tp,
                tc.tile_pool(name="h2s", bufs=2) as h2p,
            ):
                for jt2 in range(R // 128):
                    ph2 = p2p.tile([128, 18], FP32, tag="ph2")
                    for g in range(H1):
                        nc.tensor.matmul(
                            ph2[:],
                            h1T[:, g * R + jt2 * 128:g * R + (jt2 + 1) * 128],
                            W2a_sb[:, g * 18:(g + 1) * 18],
                            start=g == 0, stop=g == H1 - 1,
                        )
                    h2t = h2p.tile([128, 18], BF16, tag="h2t")
                    nc.vector.tensor_copy(h2t[:], ph2[:])
                    nc.sync.dma_start(
                        h2loc[jt2 * 128:(jt2 + 1) * 128, :], h2t[:]
                    )
                    ps2 = p2tp.tile([1, 128], BF16, tag="ps2")
                    nc.tensor.transpose(ps2[:], h2t[:, 16:17], idb_sb[:])
                    nc.vector.tensor_copy(
                        fs2row[0:1, jt2 * 128:(jt2 + 1) * 128], ps2[:]
                    )

                # local-only prep, emitted before the collective so it hides
                # inside the gather wait
                a2row = h2p.tile([1, R], BF16, tag="a2row")
                nc.scalar.activation(a2row[:], fs2row[:], AF.Copy)
                pab2 = p2tp.tile([128, R], FP32, tag="pab2")
                nc.tensor.matmul(
                    pab2[:], onesb_sb[0:1, :], a2row[0:1, :],
                    start=True, stop=True,
                )
                nc.scalar.activation(fsb2[:], pab2[:], AF.Copy)

                nc.gpsimd.collective_compute(
                    "AllGather",
                    OP.bypass,
                    replica_groups=[list(range(NCORES))],
                    ins=[h2loc[:].opt()],
                    outs=[h2all_d[:].opt()],
                )
                # quarter loads + per-quarter score factors so early chunks
                # unblock layer-2 attention before the full load lands
                QC = NCH // 4
                for qq in range(4):
                    nc.sync.dma_start(
                        h2all_sb[:, qq * QC * 18:(qq + 1) * QC * 18]
                        .rearrange("p (c o) -> p c o", o=18),
                        h2all_d[qq * QC * 128:(qq + 1) * QC * 128, :]
                        .rearrange("(c p) o -> p c o", p=128),
                    )
                    nc.vector.tensor_copy(
                        fd2_sb[:, qq * QC:(qq + 1) * QC],
                        h2all_sb[:, qq * QC * 18 + 17:(qq + 1) * QC * 18:18],
                    )
                    nc.scalar.activation(
                        vd2_sb[:, qq * QC:(qq + 1) * QC],
                        h2all_sb[:, qq * QC * 18 + 17:(qq + 1) * QC * 18:18],
                        AF.Exp, scale=ALPHA,
                    )

            # =============================================================
            # Phase E: layer-2 attention + ELU + log_softmax
            # =============================================================
            with (
                tc.tile_pool(name="acc2", bufs=1, space="PSUM") as acc2,
                tc.tile_pool(name="sc2", bufs=3) as spool2,
                tc.tile_pool(name="fin", bufs=2) as fpool,
                tc.tile_pool(name="pfin", bufs=2, space="PSUM") as pfp2,
            ):
                # overwrite the (now dead) fs2 column of gathered h2 with
                # ones: the aggregation matmul's 17th row then yields Z2.
                QC = NCH // 4
                for qq in range(4):
                    nc.vector.memset(
                        h2all_sb[:, qq * QC * 18 + 16:(qq + 1) * QC * 18:18], 1.0
                    )
                po2 = acc2.tile([17, R], FP32, tag="o2")
                for c in range(NCH):
                    t12 = spool2.tile([128, R], BF16, tag="t12")
                    nc.scalar.activation(
                        t12[:], fsb2[:], AF.Exp,
                        bias=fd2_sb[:, c:c + 1], scale=1.0 - ALPHA,
                    )
                    u2 = spool2.tile([128, R], BF16, tag="u2")
                    nc.vector.tensor_scalar_max(
                        u2[:], t12[:], vd2_sb[:, c:c + 1]
                    )
                    p2t = spool2.tile([128, R], BF16, tag="p2t")
                    nc.vector.tensor_mul(
                        p2t[:], u2[:], adjT_sb[:, c * R:(c + 1) * R]
                    )
                    nc.tensor.matmul(
                        po2[:], h2all_sb[:, c * 18:c * 18 + 17], p2t[:],
                        start=c == 0, stop=c == NCH - 1,
                    )
                po2sb = fpool.tile([17, R], FP32, tag="po2sb")
                nc.scalar.activation(po2sb[:], po2[:], AF.Copy)
                zrow = fpool.tile([1, R], FP32, tag="zrow")
                nc.sync.dma_start(zrow[0:1, :], po2sb[16:17, :])
                zrowi = fpool.tile([1, R], FP32, tag="zrowi")
                nc.vector.reciprocal_approx_fast(zrowi[:], zrow[:])
                zrowb = fpool.tile([1, R], BF16, tag="zrowb")
                nc.scalar.activation(zrowb[:], zrowi[:], AF.Copy)
                pzb = pfp2.tile([16, R], FP32, tag="pzb")
                nc.tensor.matmul(
                    pzb[:], onesb_sb[0:1, 0:16], zrowb[0:1, :],
                    start=True, stop=True,
                )
                zr2 = fpool.tile([16, R], FP32, tag="zr2")
                nc.vector.tensor_copy(zr2[:], pzb[:])
                pre2 = fpool.tile([16, R], FP32, tag="pre2")
                nc.vector.tensor_mul(pre2[:], po2sb[0:16, :], zr2[:])
                r2 = fpool.tile([16, R], FP32, tag="r2")
                nc.scalar.activation(r2[:], pre2[:], AF.Relu, scale=-1.0)
                t2 = fpool.tile([16, R], FP32, tag="t2")
                nc.scalar.activation(t2[:], r2[:], AF.Exp, scale=-1.0)
                elu2 = fpool.tile([16, R], FP32, tag="elu2")
                nc.vector.scalar_tensor_tensor(
                    elu2[:], t2[:], -1.0, pre2[:], op0=OP.add, op1=OP.max
                )
                # transpose to natural [i, o2] then log_softmax over free dim
                for it in range(R // 128):
                    pn = pfp2.tile([128, 16], FP32, tag="pn")
                    nc.tensor.transpose(
                        pn[:], elu2[:, it * 128:(it + 1) * 128], idf_sb[:]
                    )
                    nmx = fpool.tile([128, 1], FP32, tag="nmx")
                    nc.vector.tensor_reduce(
                        nmx[:], pn[:], AX.X, OP.max, negate=True
                    )
                    ex = fpool.tile([128, 16], FP32, tag="ex")
                    s = fpool.tile([128, 1], FP32, tag="s")
                    nc.scalar.activation(
                        ex[:], pn[:], AF.Exp, bias=nmx[:, 0:1], accum_out=s[:, 0:1]
                    )
                    lg = fpool.tile([128, 1], FP32, tag="lg")
                    nc.scalar.activation(lg[:], s[:], AF.Ln)
                    fin = fpool.tile([128, 16], FP32, tag="fin")
                    nc.vector.tensor_scalar(
                        fin[:], pn[:], nmx[:, 0:1], lg[:, 0:1],
                        op0=OP.add, op1=OP.subtract,
                    )
                    nc.sync.dma_start(out_d[it * 128:(it + 1) * 128, :], fin[:])

    nc.compile()
    return nc


def _get_nc():
    if "nc" not in _BUILD_CACHE:
        _BUILD_CACHE["nc"] = _build_nc()
    return _BUILD_CACHE["nc"]


def _prep_inputs(x, adj, W1, a_src1, a_dst1, W2, a_src2, a_dst2):
    bf16 = ml_dtypes.bfloat16
    f32 = np.float32
    x = np.asarray(x, f32)
    adj = np.asarray(adj, f32)
    W1 = np.asarray(W1, f32)
    W2 = np.asarray(W2, f32)
    a_src1 = np.asarray(a_src1, f32)
    a_dst1 = np.asarray(a_dst1, f32)
    a_src2 = np.asarray(a_src2, f32)
    a_dst2 = np.asarray(a_dst2, f32)

    W1f = np.ascontiguousarray(W1.reshape(F_IN, F1))
    # folded score vectors: f_src[h] = x @ (W1[:,h,:] @ a_src1[h])
    wsrc = np.stack([W1[:, h, :] @ a_src1[h] for h in range(H1)], axis=1)
    wdst = np.stack([W1[:, h, :] @ a_dst1[h] for h in range(H1)], axis=1)
    W2f = np.ascontiguousarray(W2.reshape(F1, D2))
    W2a = np.zeros((F1, 18), f32)
    W2a[:, :D2] = W2f
    W2a[:, 16] = W2f @ a_src2[0]
    W2a[:, 17] = W2f @ a_dst2[0]

    xT = np.ascontiguousarray(x.T)
    ident = np.eye(128, dtype=f32)

    shared = {
        "xT": xT.astype(bf16),
        "W1f": W1f.astype(bf16),
        "adstrow": np.ascontiguousarray(a_dst1.reshape(1, F1)).astype(bf16),
        "wsn": wsrc.astype(bf16),
        "onesb": np.ones((128, 128), bf16),
        "idb": ident.astype(bf16),
        "idf16": np.eye(16, dtype=f32),
        "W2a": W2a.astype(bf16),
    }
    in_maps = []
    for c in range(NCORES):
        blkslice = slice(c * R, (c + 1) * R)
        m = dict(shared)
        m["adjT"] = np.ascontiguousarray(adj[blkslice, :].T).astype(bf16)
        m["xTo"] = np.ascontiguousarray(x[blkslice, :].T).astype(bf16)
        in_maps.append(m)
    return in_maps


def kernel(x, adj, W1, a_src1, a_dst1, W2, a_src2, a_dst2, _trace=False):
    from concourse.bass_utils import run_bass_kernel_spmd

    nc = _get_nc()
    in_maps = _prep_inputs(x, adj, W1, a_src1, a_dst1, W2, a_src2, a_dst2)
    res = run_bass_kernel_spmd(nc, in_maps, list(range(NCORES)), trace=_trace)
    out = np.concatenate(
        [np.asarray(res.results[c]["out"]) for c in range(NCORES)], axis=0
    )
    kernel.last_results = res
    return out.astype(np.float32)
